# revision 38
# baseline (speedup 1.0000x reference)
"""Trainium2 Bass kernel for nn_Attention_80092550136278.

Gated attention with pair bias:
  q = (q_data @ Wq) * d^-0.5 ; k = k_data @ Wk ; v = v_data @ Wv   (per head)
  w = softmax(q k^T + pair_bias) ; ctx = w @ v
  out = (ctx * sigmoid(q_data @ Wg.T + gating_b)) @ Wo.T + o_bias

Sharding (v3): 2D — 4 q-shards x 2 head-groups over 8 cores. Core c handles
head group g = c % 2 (4 heads) and q rows s = c // 2 (512 rows). Outputs of
the two head-group cores of each q-slice are summed on the host during the
gather (the "all-reduce after the output projection" of head parallelism);
o_bias is zeroed on g=1 cores so it is added exactly once.

Logits are computed TRANSPOSED, [k, q]:
  logits^T[k, q] = (k_projT chunk)[d=32, 128k]^T @ q_projT[d=32, 512q]
with pair_bias pre-transposed on the host to [kc, 128k, 4h*512q] bf16. This
removes the PE transposes of v1 and feeds softmax(w)^T straight into the
ctx matmul. With N=512, every K=32 logits matmul writes exactly one full
PSUM bank at offset 0 — avoiding a HW lockup observed when a matmul with a
nonzero tile_position row offset writes PSUM at a nonzero bank offset.

Pipeline (kc = 16 chunks of 128 k-rows, halves = head pairs):
  DMA bias half-tile [128, 2h*512q] (sync/HWDGE)
  PE: 2 logits MMs K=32 N=512 at tile_position rows (h%4)*32 (concurrent
      across consecutive heads)
  DVE: pl += bias (in-place in PSUM, fp32)
  ACT: wT = exp(pl) -> SBUF bf16
  PE: 2 ctx MMs (K=128, v_aug weights with a ones column) accumulate
      pctx[h] [33, 512] — one full PSUM bank per head — across all 16 kc;
      row 32 accumulates the softmax denominators.
Tail: reciprocal of denominators (DVE fast-approx at partition 0, broadcast
via ones outer-product MM), comb = ctx * gate * recip, out-partial =
comb @ Wo_g^T (+ o_bias on g=0), DMA out.

All heavy tensors are bf16 (inputs/weights cast on host): halves HBM
traffic (bias is 8.4 MB/core). Accumulation stays fp32 in PSUM.
"""

import numpy as np

H, D, NQT, NK, C = 8, 32, 2048, 2048, 256
NQ = 512               # q rows per core (4 q-shards)
HG = 4                 # heads per core (2 head groups)
KC = NK // 128         # 16 k-chunks
SCALE = D ** -0.5

_CACHE = {}
DEBUG_DUMPS = False
CTX_INTERLEAVED = True


def _build_nc():
    import concourse.bass as bass
    import concourse.bacc as bacc
    import concourse.tile as tile
    import concourse.mybir as mybir

    F32 = mybir.dt.float32
    F32R = mybir.dt.float32r
    BF16 = mybir.dt.bfloat16
    AF = mybir.ActivationFunctionType

    nc = bacc.Bacc("TRN2", debug=False)

    # ---- DRAM I/O (per core: head group g, q slice s) ----
    d_qT = nc.dram_tensor("qT", [C, NQ], BF16, kind="ExternalInput")
    d_kT = nc.dram_tensor("kT", [C, NK], BF16, kind="ExternalInput")
    d_vT = nc.dram_tensor("vT", [C, NK], BF16, kind="ExternalInput")
    d_biasT = nc.dram_tensor("biasT", [KC, 128, HG * NQ], BF16, kind="ExternalInput")
    d_wq = nc.dram_tensor("wq", [C, 128], BF16, kind="ExternalInput")
    d_wk = nc.dram_tensor("wk", [C, 128], BF16, kind="ExternalInput")
    d_wv = nc.dram_tensor("wv", [C, 128], BF16, kind="ExternalInput")
    d_wgT = nc.dram_tensor("wgT", [C, 128], BF16, kind="ExternalInput")
    d_woT = nc.dram_tensor("woT", [D, HG * C], F32R, kind="ExternalInput")
    d_gb = nc.dram_tensor("gb", [D, HG], F32, kind="ExternalInput")
    d_ob = nc.dram_tensor("ob", [1, C], F32R, kind="ExternalInput")
    d_ones = nc.dram_tensor("ones", [128, 128], F32R, kind="ExternalInput")
    d_id = nc.dram_tensor("ident", [128, 128], BF16, kind="ExternalInput")
    d_out = nc.dram_tensor("out", [NQ, C], F32, kind="ExternalOutput")
    if DEBUG_DUMPS:
        d_dbg_qp = nc.dram_tensor("dbg_qp", [128, NQ], BF16, kind="ExternalOutput")
        d_dbg_kp = nc.dram_tensor("dbg_kp", [128, NK], BF16, kind="ExternalOutput")
        d_dbg_va = nc.dram_tensor("dbg_va", [128, KC * HG * 33], BF16, kind="ExternalOutput")
        d_dbg_wT = nc.dram_tensor("dbg_wT", [2, 128, 2 * NQ], BF16, kind="ExternalOutput")
        d_dbg_comb = nc.dram_tensor("dbg_comb", [D, HG * NQ], F32, kind="ExternalOutput")

    with tile.TileContext(nc) as tc:
        with tc.tile_pool(name="persist", bufs=1) as pers:

            # ---------------- persistent SBUF ----------------
            q_projT = pers.tile([128, NQ], BF16, name="q_projT")
            k_projT = [pers.tile([128, 512], BF16, name=f"k_projT{i}")
                       for i in range(4)]
            v_aug = [pers.tile([128, 4 * HG * 33], BF16, name=f"v_aug{i}")
                     for i in range(4)]
            gate_sb = pers.tile([D, HG * NQ], F32, name="gate_sb")
            comb = pers.tile([D, HG * NQ], F32R, name="comb")
            woT_sb = pers.tile([D, HG * C], F32R, name="woT_sb")
            gb_sb = pers.tile([D, HG], F32, name="gb_sb")
            ob_sb = pers.tile([1, C], F32R, name="ob_sb")
            ones_sb = pers.tile([128, 128], F32R, name="ones_sb")
            id_sb = pers.tile([128, 128], BF16, name="id_sb")
            rsr = pers.tile([33, HG * NQ], F32, name="rsr")
            denT = pers.tile([33, HG * NQ], F32R, name="denT")
            out_sb = [pers.tile([128, C], F32, name=f"out_sb{i}") for i in range(4)]

            # bias pool allocated before stage 1 so early-kc prefetch overlaps
            # the projections; one [128, 2*NQ] tile per half-kc
            bias_pool = tc.alloc_tile_pool(name="bias_sb", bufs=10)
            bias_pre = []

            with tc.tile_pool(name="stage1_sb", bufs=1) as s1, \
                 tc.tile_pool(name="stage1_ps", bufs=4, space="PSUM") as pp:
                qT_sb = [s1.tile([128, NQ], BF16, name=f"qT{i}") for i in range(2)]
                kT_sb = [s1.tile([128, NK], BF16, name=f"kT{i}") for i in range(2)]
                vT_sb = [s1.tile([128, NK], BF16, name=f"vT{i}") for i in range(2)]
                wq_sb = [s1.tile([128, 128], BF16, name=f"wq{i}") for i in range(2)]
                wk_sb = [s1.tile([128, 128], BF16, name=f"wk{i}") for i in range(2)]
                wv_sb = [s1.tile([128, 128], BF16, name=f"wv{i}") for i in range(2)]
                wgT_sb = [s1.tile([128, 128], BF16, name=f"wgT{i}") for i in range(2)]
                # critical-path first: q/wq (q_proj), wgT (gate), wk, then
                # the rest; bulk kT/vT lead the sync queue, then bias prefetch
                for i in range(2):
                    sl = slice(i * 128, (i + 1) * 128)
                    nc.scalar.dma_start(qT_sb[i][:], d_qT.ap()[sl, :])
                    nc.scalar.dma_start(wq_sb[i][:], d_wq.ap()[sl, :])
                    nc.scalar.dma_start(wgT_sb[i][:], d_wgT.ap()[sl, :])
                for i in range(2):
                    sl = slice(i * 128, (i + 1) * 128)
                    for ch in range(2):
                        cs2 = slice(ch * 1024, (ch + 1) * 1024)
                        nc.sync.dma_start(kT_sb[i][:, cs2], d_kT.ap()[sl, cs2])
                for i in range(2):
                    sl = slice(i * 128, (i + 1) * 128)
                    nc.scalar.dma_start(wk_sb[i][:], d_wk.ap()[sl, :])
                    nc.scalar.dma_start(wv_sb[i][:], d_wv.ap()[sl, :])
                for i in range(2):
                    sl = slice(i * 128, (i + 1) * 128)
                    for ch in range(2):
                        cs2 = slice(ch * 1024, (ch + 1) * 1024)
                        nc.sync.dma_start(vT_sb[i][:, cs2], d_vT.ap()[sl, cs2])
                nc.gpsimd.dma_start(id_sb[:], d_id.ap()[:])
                nc.gpsimd.dma_start(gb_sb[:], d_gb.ap()[:])
                nc.gpsimd.dma_start(ones_sb[:], d_ones.ap()[:])
                nc.gpsimd.dma_start(woT_sb[:], d_woT.ap()[:])
                nc.gpsimd.dma_start(ob_sb[:], d_ob.ap()[:])
                for pre in range(4):
                    bt = bias_pool.tile([128, 2 * NQ], BF16, tag="bias",
                                        name="bias_t")
                    base = (pre % 2) * 2 * NQ
                    if pre < 2:
                        for ch in range(2):
                            nc.sync.dma_start(
                                bt[:, ch * NQ:(ch + 1) * NQ],
                                d_biasT.ap()[pre // 2, :,
                                             base + ch * NQ:base + (ch + 1) * NQ])
                    else:
                        nc.sync.dma_start(bt[:], d_biasT.ap()[pre // 2, :,
                                                              base:base + 2 * NQ])
                    bias_pre.append(bt)

                # ones columns of v_aug (position 32 of each 33-wide block)
                v_aug4w = [v_aug[i].rearrange("p (n h e) -> p n h e", n=4, h=HG)
                           for i in range(4)]
                for i in range(4):
                    nc.vector.tensor_copy(
                        v_aug4w[i][:, :, :, D:D + 1],
                        ones_sb[:, 0:4 * HG].bitcast(F32).rearrange(
                            "p (n h) -> p n h", n=4).unsqueeze(-1))

                # q_projT [128, NQ] = (Wq_g.T @ q_data.T), rows = 4h x 32d
                pq = pp.tile([128, NQ], F32, tag="proj", name="pq")
                for kk in range(2):
                    nc.tensor.matmul(pq[:], wq_sb[kk][:], qT_sb[kk][:],
                                     start=(kk == 0), stop=(kk == 1))
                nc.vector.tensor_copy(q_projT[:], pq[:])

                # gate per head: sigmoid(WgT_h.T @ qT + gb_h), [32, NQ]
                for h in range(HG):
                    pg = pp.tile([D, NQ], F32, tag="proj", name="pg")
                    for kk in range(2):
                        nc.tensor.matmul(pg[:], wgT_sb[kk][:, h * D:(h + 1) * D],
                                         qT_sb[kk][:], start=(kk == 0), stop=(kk == 1))
                    nc.scalar.activation(gate_sb[:, h * NQ:(h + 1) * NQ], pg[:],
                                         AF.Sigmoid, bias=gb_sb[:, h:h + 1])

                # k_projT [128, NK]
                for nn in range(NK // 512):
                    pk = pp.tile([128, 512], F32, tag="proj", name="pk")
                    for kk in range(2):
                        nc.tensor.matmul(pk[:], wk_sb[kk][:],
                                         kT_sb[kk][:, nn * 512:(nn + 1) * 512],
                                         start=(kk == 0), stop=(kk == 1))
                    nc.vector.tensor_copy(k_projT[nn][:], pk[:])

                # v_proj natural layout -> scatter into v_aug (4 nn per psum)
                for nb in range(4):
                    pv = pp.tile([128, 512], F32, tag="proj", name="pv")
                    for j in range(4):
                        nn = nb * 4 + j
                        for kk in range(2):
                            nc.tensor.matmul(pv[:, j * 128:(j + 1) * 128],
                                             vT_sb[kk][:, nn * 128:(nn + 1) * 128],
                                             wv_sb[kk][:], start=(kk == 0), stop=(kk == 1))
                    nc.scalar.copy(
                        v_aug4w[nb][:, :, :, 0:D],
                        pv[:].rearrange("p (n h d) -> p n h d", n=4, h=HG))

            if DEBUG_DUMPS:
                nc.sync.dma_start(d_dbg_qp.ap()[:], q_projT[:])
                for i in range(4):
                    nc.sync.dma_start(d_dbg_kp.ap()[:, i * 512:(i + 1) * 512],
                                      k_projT[i][:])
                    nc.sync.dma_start(
                        d_dbg_va.ap()[:, i * 4 * HG * 33:(i + 1) * 4 * HG * 33],
                        v_aug[i][:])

            # ---------------- stage 2+3: attention ----------------
            wT_bufs = 6 if CTX_INTERLEAVED else 32
            wT_pool = tc.alloc_tile_pool(name="wT_sb", bufs=wT_bufs)
            s_pool = tc.alloc_tile_pool(name="s_sb", bufs=6)
            pl_pool = tc.alloc_tile_pool(name="pl", bufs=2, space="PSUM")
            pctx_pool = tc.alloc_tile_pool(name="pctx", bufs=1, space="PSUM")
            pctx = [pctx_pool.tile([128, 2 * NQ], F32, name=f"pctx{i}") for i in range(2)]
            wT_tiles = []

            # software-pipelined emission: the front stage (bias DMA +
            # inject/logits matmuls into pl) runs one half-iteration ahead of
            # the back stage (add/exp + ctx matmuls), so ready logits MMs are
            # not stuck behind exp-dependent ctx MMs in the PE queue.
            NHALF = KC * 2
            front = {}

            def emit_front(idx):
                kc, half = idx // 2, idx % 2
                inject = (idx % 2 == 0)
                if idx < 4:
                    bias_t = bias_pre[idx]
                else:
                    bias_t = bias_pool.tile([128, 2 * NQ], BF16, tag="bias",
                                            name="bias_t")
                    nc.sync.dma_start(
                        bias_t[:],
                        d_biasT.ap()[kc, :, half * 2 * NQ:(half + 1) * 2 * NQ])
                pl = pl_pool.tile([128, 2 * NQ], F32, tag="pl", name="pl")
                if inject:
                    # bias -> PSUM via identity matmul; logits accumulate
                    for j in range(2):
                        nc.tensor.matmul(pl[:, j * NQ:(j + 1) * NQ], id_sb[:],
                                         bias_t[:, j * NQ:(j + 1) * NQ],
                                         start=True, stop=False)
                for j in range(2):
                    h = half * 2 + j
                    po = h * 32
                    nc.tensor.matmul(pl[:, j * NQ:(j + 1) * NQ],
                                     k_projT[kc // 4][po:po + 32,
                                                     (kc % 4) * 128:(kc % 4) * 128 + 128],
                                     q_projT[po:po + 32, :],
                                     start=(not inject), stop=True,
                                     tile_position=(po, 0))
                front[idx] = (bias_t, pl)

            def emit_back(idx):
                kc, half = idx // 2, idx % 2
                inject = (idx % 2 == 0)
                bias_t, pl = front.pop(idx)
                wT = wT_pool.tile([128, 2 * NQ], BF16, tag="wT", name="wT")
                if inject:
                    # exp straight from PSUM (bank held until exp retires)
                    nc.scalar.activation(wT[:], pl[:], AF.Exp)
                else:
                    # add releases the PSUM bank early; exp reads SBUF
                    s_t = s_pool.tile([128, 2 * NQ], BF16, tag="s", name="s_t")
                    nc.vector.tensor_add(s_t[:], pl[:], bias_t[:])
                    nc.scalar.activation(wT[:], s_t[:], AF.Exp)
                if DEBUG_DUMPS and kc == 0:
                    nc.sync.dma_start(d_dbg_wT.ap()[half], wT[:])
                if CTX_INTERLEAVED:
                    for j in range(2):
                        h = half * 2 + j
                        co = ((kc % 4) * HG + h) * 33
                        nc.tensor.matmul(pctx[h // 2][0:33, (h % 2) * NQ:(h % 2) * NQ + NQ],
                                         v_aug[kc // 4][:, co:co + 33],
                                         wT[:, j * NQ:(j + 1) * NQ],
                                         start=(kc == 0), stop=(kc == KC - 1))
                else:
                    wT_tiles.append(wT)

            for kc2 in range(KC):
                emit_front(2 * kc2)
                emit_front(2 * kc2 + 1)
                emit_back(2 * kc2)
                emit_back(2 * kc2 + 1)

            if not CTX_INTERLEAVED:
                for h in range(HG):
                    for kc in range(KC):
                        co = ((kc % 4) * HG + h) * 33
                        wT = wT_tiles[kc * 2 + h // 2]
                        nc.tensor.matmul(pctx[h // 2][0:33, (h % 2) * NQ:(h % 2) * NQ + NQ],
                                         v_aug[kc // 4][:, co:co + 33],
                                         wT[:, (h % 2) * NQ:(h % 2) * NQ + NQ],
                                         start=(kc == 0), stop=(kc == KC - 1))

            # ---------------- tail: normalize, gate, output ----------------
            for hp in range(2):
                cs = pctx[hp][0:33, :]
                sl2 = slice(hp * 2 * NQ, (hp + 1) * 2 * NQ)
                nc.vector.tensor_copy(denT[32:33, sl2], cs[32:33, :])
                prsb = pl_pool.tile([128, 2 * NQ], F32, tag="pl", name="prsb")
                for j in range(2):
                    nc.tensor.matmul(prsb[0:32, j * NQ:(j + 1) * NQ],
                                     ones_sb[32:33, 0:32],
                                     denT[32:33, (hp * 2 + j) * NQ:(hp * 2 + j + 1) * NQ],
                                     start=True, stop=True, tile_position=(32, 0))
                nc.vector.reciprocal_approx_fast(
                    out=rsr[0:32, sl2], in_=prsb[0:32, 0:2 * NQ])
                csl = comb[:, sl2]
                nc.vector.tensor_mul(csl, cs[0:32, :], gate_sb[:, sl2])
                nc.vector.tensor_mul(csl, csl.bitcast(F32), rsr[0:32, sl2])

            if DEBUG_DUMPS:
                nc.sync.dma_start(d_dbg_comb.ap()[:], comb[:].bitcast(F32))

            # out-partial[512, 256] = sum_h comb_h^T @ WoT_h (+ ones^T x o_bias)
            for qm in range(4):
                pout = pl_pool.tile([128, 2 * NQ], F32, tag="pl", name="pout")
                for h in range(HG):
                    nc.tensor.matmul(pout[:, 0:C],
                                     comb[:, h * NQ + qm * 128: h * NQ + qm * 128 + 128],
                                     woT_sb[:, h * C:(h + 1) * C],
                                     start=(h == 0), stop=False)
                nc.tensor.matmul(pout[:, 0:C], ones_sb[0:1, 0:128], ob_sb[:],
                                 start=False, stop=True)
                nc.scalar.copy(out_sb[qm][:], pout[:, 0:C])
                nc.sync.dma_start(d_out.ap()[qm * 128:qm * 128 + 64, :],
                                  out_sb[qm][0:64, :])
                nc.sync.dma_start(d_out.ap()[qm * 128 + 64:qm * 128 + 128, :],
                                  out_sb[qm][64:128, :])

            pctx_pool.release()
            pl_pool.release()
            s_pool.release()
            wT_pool.release()
            bias_pool.release()

    nc.compile()
    return nc


def _prep_in_maps(inputs):
    import ml_dtypes
    BF = ml_dtypes.bfloat16
    q_data = np.asarray(inputs["q_data"], dtype=np.float32)
    k_data = np.asarray(inputs["k_data"], dtype=np.float32)
    v_data = np.asarray(inputs["v_data"], dtype=np.float32)
    pair_bias = np.asarray(inputs["pair_bias"], dtype=np.float32)
    Wq = np.asarray(inputs["Wq"], dtype=np.float32)
    Wk = np.asarray(inputs["Wk"], dtype=np.float32)
    Wv = np.asarray(inputs["Wv"], dtype=np.float32)
    Wg = np.asarray(inputs["Wg"], dtype=np.float32)
    Wo = np.asarray(inputs["Wo"], dtype=np.float32)
    gating_b = np.asarray(inputs["gating_b"], dtype=np.float32)
    o_bias = np.asarray(inputs["o_bias"], dtype=np.float32)

    kT = np.ascontiguousarray(k_data.T).astype(BF)
    vT = np.ascontiguousarray(v_data.T).astype(BF)
    wq_s = Wq * np.float32(SCALE)
    wgT = np.ascontiguousarray(Wg.T)
    woT = Wo.T.reshape(H, D, C)
    gbT = gating_b.T                      # [D, H]
    ones = np.ones((128, 128), dtype=np.float32)
    ident = np.eye(128, dtype=np.float32).astype(BF)
    # [k, h, q] bias, bf16, shared across cores (cores slice heads + q rows)
    pbT = np.ascontiguousarray(pair_bias.transpose(2, 0, 1)).astype(BF)
    ob = o_bias.reshape(1, C)
    ob0 = np.zeros_like(ob)

    in_maps = []
    for c in range(8):
        g, s = c % 2, c // 2
        hsl = slice(g * HG, (g + 1) * HG)
        csl = slice(g * 128, (g + 1) * 128)
        qs = slice(s * NQ, (s + 1) * NQ)
        biasT = np.ascontiguousarray(pbT[:, hsl, qs]).reshape(KC, 128, HG * NQ)
        in_maps.append(dict(
            qT=np.ascontiguousarray(q_data[qs, :].T).astype(BF),
            kT=kT, vT=vT, biasT=biasT,
            wq=np.ascontiguousarray(wq_s[:, csl]).astype(BF),
            wk=np.ascontiguousarray(Wk[:, csl]).astype(BF),
            wv=np.ascontiguousarray(Wv[:, csl]).astype(BF),
            wgT=np.ascontiguousarray(wgT[:, csl]).astype(BF),
            woT=np.ascontiguousarray(
                woT[hsl].transpose(1, 0, 2).reshape(D, HG * C)),
            gb=np.ascontiguousarray(gbT[:, hsl]),
            ob=(ob if g == 0 else ob0),
            ones=ones, ident=ident,
        ))
    return in_maps


def _get_nc():
    if "nc" not in _CACHE:
        _CACHE["nc"] = _build_nc()
    return _CACHE["nc"]


def _run(inputs, trace=False, trace_cores=None):
    from concourse import bass_utils
    nc = _get_nc()
    in_maps = _prep_in_maps(inputs)
    kwargs = {}
    if trace:
        kwargs = dict(trace=True, trace_cores=trace_cores or [0])
    res = bass_utils.run_bass_kernel_spmd(nc, in_maps, core_ids=list(range(8)), **kwargs)
    # gather: sum the two head-group partials of each q slice
    out = np.concatenate(
        [res.results[2 * s]["out"] + res.results[2 * s + 1]["out"] for s in range(4)],
        axis=0)
    return out, res


def kernel(**inputs) -> np.ndarray:
    out, _ = _run(inputs)
    return out
